# revision 1
# baseline (speedup 1.0000x reference)
"""GNN message-passing kernel for trn2: preprocessing + bass/tile builder."""
import numpy as np
import ml_dtypes
import concourse.bass as bass
import concourse.tile as tile
from concourse import bacc, mybir
from concourse.bass_utils import run_bass_kernel_spmd

F32 = mybir.dt.float32
BF16 = mybir.dt.bfloat16
I16 = mybir.dt.int16
P = 128


def preprocess(x, edge_index, batch, NC=8, QUAD=32768, table_np=ml_dtypes.bfloat16):
    """Host-side graph preprocessing. Returns (struct, per_core_common, meta)."""
    x = np.asarray(x, np.float32)
    ei = np.asarray(edge_index, np.int64)
    b = np.asarray(batch, np.int64)
    N = x.shape[0]
    G = int(b.max()) + 1
    assert G % NC == 0, (G, NC)
    GPC = G // NC
    counts = np.bincount(b, minlength=G)
    assert counts.min() > 0
    WPG = int(np.ceil(counts.max() / P))  # windows per graph
    NPG = WPG * P
    WIN = GPC * WPG                      # windows per core
    NPC = WIN * P                        # padded nodes per core
    NTOT = NC * NPC
    NQ = int(np.ceil(NTOT / QUAD))

    # node permutation: graph g -> core g//GPC, slot (g%GPC)*NPG + j
    cum = np.concatenate([[0], np.cumsum(counts)])
    base_new = (np.arange(G) // GPC) * NPC + (np.arange(G) % GPC) * NPG
    perm = base_new[b] + (np.arange(N) - cum[b])     # orig id -> new id

    xt = np.zeros((NTOT, x.shape[1]), table_np)
    xt[perm] = x.astype(table_np)

    src = perm[ei[0]]
    dst = perm[ei[1]]
    deg = np.bincount(dst, minlength=NTOT)
    recip_full = (1.0 / np.maximum(deg, 1)).astype(np.float32)
    mask_full = (deg > 0).astype(np.float32)

    core = dst // NPC
    w = (dst % NPC) // P
    dl = (dst % P).astype(np.int64)
    q = src // QUAD
    key = ((core * WIN + w) * NQ + q).astype(np.int64)
    order = np.argsort(key, kind="stable")
    s_src = src[order]
    s_dl = dl[order]
    s_key = key[order]
    L = np.bincount(s_key, minlength=NC * WIN * NQ).reshape(NC, WIN, NQ)
    S = np.ceil(L / P).astype(np.int64).max(axis=0)  # [WIN, NQ] subtiles
    S_tot = int(S.sum())
    S_w = S.sum(axis=1)  # [WIN]

    # group windows for batched gathers; subtile order: (group, q, w in group)
    GRP = 4
    NG = int(np.ceil(WIN / GRP))
    sub0 = np.zeros((WIN, NQ), np.int64)
    run = 0
    for g in range(NG):
        ws = range(g * GRP, min((g + 1) * GRP, WIN))
        for qi in range(NQ):
            for wi in ws:
                sub0[wi, qi] = run
                run += S[wi, qi]
    assert run == S_tot

    # per-core edge slot arrays
    idx_flat = np.zeros((NC, S_tot * P), np.int64)       # pad -> index 0
    dl_flat = np.full((NC, S_tot * P), 255, np.int64)    # pad -> dead dst
    grp_start = np.concatenate([[0], np.cumsum(L.reshape(-1))])
    for c in range(NC):
        for wi in range(WIN):
            for qi in range(NQ):
                g = (c * WIN + wi) * NQ + qi
                n = L[c, wi, qi]
                if n == 0:
                    continue
                a = grp_start[g]
                base = sub0[wi, qi] * P
                idx_flat[c, base:base + n] = s_src[a:a + n] - qi * QUAD
                dl_flat[c, base:base + n] = s_dl[a:a + n]
    assert idx_flat.max() < QUAD and idx_flat.min() >= 0

    # wrap indices into 16 partitions: flat j -> [j%16, j//16]; replicate to 128
    idx16 = np.ascontiguousarray(
        idx_flat.reshape(NC, S_tot * 8, 16).transpose(0, 2, 1)).astype(np.int16)
    idx16 = np.tile(idx16, (1, 8, 1))                    # [NC, 128, S_tot*8]
    # dst per subtile: [128, S_tot]
    dst_arr = np.ascontiguousarray(
        dl_flat.reshape(NC, S_tot, P).transpose(0, 2, 1)).astype(ml_dtypes.bfloat16)

    recip_pc = recip_full.reshape(NC, WIN, P).transpose(0, 2, 1).copy()  # [NC,128,WIN]
    mask_pc = mask_full.reshape(NC, 1, NPC).astype(ml_dtypes.bfloat16)   # [NC,1,NPC]

    struct = dict(NC=NC, G=G, GPC=GPC, WPG=WPG, WIN=WIN, NPC=NPC, NTOT=NTOT,
                  NQ=NQ, QUAD=QUAD, S=S, sub0=sub0, S_tot=S_tot, S_w=S_w,
                  GRP=GRP, NG=NG)
    percore = dict(idx16=idx16, dst=dst_arr, recip=recip_pc, mask=mask_pc, xt=xt)
    pad_frac = S_tot * P / max(len(s_src) / NC, 1) - 1
    meta = dict(pad_frac=pad_frac, WPG=WPG, S_tot=S_tot)
    return struct, percore, meta


def build_nc(st, D=128, OUT=2, table_dt=BF16, debug_taps=False):
    NC, WIN, NPC, NTOT, NQ, QUAD = st["NC"], st["WIN"], st["NPC"], st["NTOT"], st["NQ"], st["QUAD"]
    S, sub0, S_tot, GPC, WPG = st["S"], st["sub0"], st["S_tot"], st["GPC"], st["WPG"]
    DT = BF16  # compute dtype for aggregation path

    nc = bacc.Bacc("TRN2", target_bir_lowering=False, debug=False,
                   num_devices=NC, num_swdge_queues=4,
                   dynamic_dma_scratch_size=16384)
    xt = nc.dram_tensor("xt", [NTOT, D], table_dt, kind="ExternalInput")
    idx_in = nc.dram_tensor("idx16", [P, S_tot * 8], I16, kind="ExternalInput")
    dst_in = nc.dram_tensor("dstl", [P, S_tot], BF16, kind="ExternalInput")
    recip_in = nc.dram_tensor("recip", [P, WIN], F32, kind="ExternalInput")
    mask_in = nc.dram_tensor("mask", [1, NPC], BF16, kind="ExternalInput")
    w1t_in = nc.dram_tensor("w1t", [D, D], DT, kind="ExternalInput")
    w2t_in = nc.dram_tensor("w2t", [D, D], DT, kind="ExternalInput")
    b1r_in = nc.dram_tensor("b1r", [1, D], DT, kind="ExternalInput")
    b2r_in = nc.dram_tensor("b2r", [1, D], DT, kind="ExternalInput")
    wf1t_in = nc.dram_tensor("wf1t", [D, D], F32, kind="ExternalInput")
    bf1r_in = nc.dram_tensor("bf1r", [1, D], F32, kind="ExternalInput")
    wf2t_in = nc.dram_tensor("wf2t", [D, OUT], F32, kind="ExternalInput")
    bf2r_in = nc.dram_tensor("bf2r", [1, OUT], F32, kind="ExternalInput")
    iota_in = nc.dram_tensor("iota", [P, P], BF16, kind="ExternalInput")
    identb_in = nc.dram_tensor("identb", [P, P], BF16, kind="ExternalInput")
    identf_in = nc.dram_tensor("identf", [P, P], F32, kind="ExternalInput")
    onesg_in = nc.dram_tensor("onesg", [1, NC * GPC], F32, kind="ExternalInput")
    out = nc.dram_tensor("out", [NC * GPC, OUT], F32, kind="ExternalOutput")
    if debug_taps:
        dbg_h = nc.dram_tensor("dbg_h", [NPC, D], F32, kind="ExternalOutput")
        dbg_pool = nc.dram_tensor("dbg_pool", [P, NC * GPC], F32, kind="ExternalOutput")

    with tile.TileContext(nc) as tc:
        cp = tc.alloc_tile_pool(name="const", bufs=1)
        wp = tc.alloc_tile_pool(name="work", bufs=3)
        mp = tc.alloc_tile_pool(name="msgs", bufs=2)
        ohp = tc.alloc_tile_pool(name="ohp", bufs=4)
        pp_agg = tc.alloc_tile_pool(name="ps_agg", bufs=2, space="PSUM")
        pp_t = tc.alloc_tile_pool(name="ps_t", bufs=2, space="PSUM")
        pp_h = tc.alloc_tile_pool(name="ps_h", bufs=2, space="PSUM")
        pp_p = tc.alloc_tile_pool(name="ps_p", bufs=2, space="PSUM")
        dp = tc.alloc_tile_pool(name="dram", bufs=1, space="DRAM")

        h_loc = dp.tile([NPC, D], table_dt)
        h_tab = dp.tile([NTOT, D], table_dt, addr_space="Shared")
        pag_in = dp.tile([P, GPC], F32)
        pag_out = dp.tile([NC * P, GPC], F32, addr_space="Shared")

        # constants
        idx_t = cp.tile([P, S_tot * 8], I16)
        nc.sync.dma_start(idx_t[:], idx_in[:])
        dst_t = cp.tile([P, S_tot], BF16)
        nc.sync.dma_start(dst_t[:], dst_in[:])
        recip_t = cp.tile([P, WIN], F32)
        nc.sync.dma_start(recip_t[:], recip_in[:])
        mask_t = cp.tile([1, NPC], BF16)
        nc.sync.dma_start(mask_t[:], mask_in[:])
        w1t_t = cp.tile([D, D], DT)
        nc.sync.dma_start(w1t_t[:], w1t_in[:])
        w2t_t = cp.tile([D, D], DT)
        nc.sync.dma_start(w2t_t[:], w2t_in[:])
        b1r_t = cp.tile([1, D], DT)
        nc.sync.dma_start(b1r_t[:], b1r_in[:])
        b2r_t = cp.tile([1, D], DT)
        nc.sync.dma_start(b2r_t[:], b2r_in[:])
        wf1t_t = cp.tile([D, D], F32)
        nc.sync.dma_start(wf1t_t[:], wf1t_in[:])
        bf1r_t = cp.tile([1, D], F32)
        nc.sync.dma_start(bf1r_t[:], bf1r_in[:])
        wf2t_t = cp.tile([D, OUT], F32)
        nc.sync.dma_start(wf2t_t[:], wf2t_in[:])
        bf2r_t = cp.tile([1, OUT], F32)
        nc.sync.dma_start(bf2r_t[:], bf2r_in[:])
        iota_t = cp.tile([P, P], BF16)
        nc.sync.dma_start(iota_t[:], iota_in[:])
        identb_t = cp.tile([P, P], BF16)
        nc.sync.dma_start(identb_t[:], identb_in[:])
        identf_t = cp.tile([P, P], F32)
        nc.sync.dma_start(identf_t[:], identf_in[:])
        onesg_t = cp.tile([1, NC * GPC], F32)
        nc.sync.dma_start(onesg_t[:], onesg_in[:])
        zero_t = cp.tile([P, D], table_dt)
        nc.vector.memset(zero_t[:], 0)
        poolT = cp.tile([P, GPC], F32)
        nc.vector.memset(poolT[:], 0)

        qrows = [min(QUAD, NTOT - qi * QUAD) for qi in range(NQ)]

        for layer in range(2):
            table = xt if layer == 0 else h_tab
            wt = w1t_t if layer == 0 else w2t_t
            br = b1r_t if layer == 0 else b2r_t
            GRP, NG = st["GRP"], st["NG"]
            S_grp = np.zeros((NG, NQ), np.int64)
            for g in range(NG):
                for qi in range(NQ):
                    S_grp[g, qi] = S[g * GRP:(g + 1) * GRP, qi].sum()
            gbase = np.zeros(NG, np.int64)
            for g in range(1, NG):
                gbase[g] = gbase[g - 1] + S_grp[g - 1].sum()
            Sg_max = int(S_grp.sum(axis=1).max())
            for g in range(NG):
                Sg = int(S_grp[g].sum())
                ws = list(range(g * GRP, min((g + 1) * GRP, WIN)))
                if Sg > 0:
                    msgs = mp.tile([P, Sg * D], table_dt, tag="msgs",
                                   padded_shape=[P, Sg_max * D])
                    m3 = msgs[:].rearrange("p (s d) -> p s d", d=D)
                    MAXSUB = 8  # <=1024 idx per call (16KB swdge scratch)
                    off = 0
                    for qi in range(NQ):
                        Sq = int(S_grp[g, qi])
                        done = 0
                        while done < Sq:
                            n = min(MAXSUB, Sq - done)
                            c0 = (int(gbase[g]) + off) * 8
                            nc.gpsimd.dma_gather(
                                out_ap=m3[:, off:off + n, :],
                                in_ap=table[qi * QUAD: qi * QUAD + qrows[qi], :],
                                idxs_ap=idx_t[:, c0: c0 + n * 8],
                                num_idxs=n * P,
                                num_idxs_reg=n * P,
                                elem_size=D,
                                queue_num=qi % 4,
                            )
                            off += n
                            done += n
                for w in ws:
                    Sw = int(st["S_w"][w])
                    if Sw == 0:
                        if layer == 0:
                            nc.sync.dma_start(h_loc[w * P:(w + 1) * P, :], zero_t[:])
                        continue
                    agg_p = pp_agg.tile([P, D], F32, tag="agg")
                    si = 0
                    for qi in range(NQ):
                        for s in range(int(S[w, qi])):
                            gs = int(sub0[w, qi]) + s
                            ms = gs - int(gbase[g])
                            oh = ohp.tile([P, P], BF16, tag="oh")
                            nc.vector.tensor_tensor(
                                out=oh[:],
                                in0=dst_t[:, gs:gs + 1].to_broadcast([P, P]),
                                in1=iota_t[:],
                                op=mybir.AluOpType.is_equal,
                            )
                            nc.tensor.matmul(agg_p[:], lhsT=oh[:], rhs=m3[:, ms, :],
                                             start=(si == 0), stop=(si == Sw - 1))
                            si += 1
                    agg_s = wp.tile([P, D], BF16, tag="aggs")
                    nc.vector.tensor_scalar(out=agg_s[:], in0=agg_p[:],
                                            scalar1=recip_t[:, w:w + 1], scalar2=None,
                                            op0=mybir.AluOpType.mult)
                    aggT_p = pp_t.tile([P, D], BF16, tag="aggT")
                    nc.tensor.transpose(aggT_p[:], agg_s[:], identb_t[:])
                    aggT_s = wp.tile([P, D], BF16, tag="aggTs")
                    nc.scalar.activation(aggT_s[:], aggT_p[:],
                                         mybir.ActivationFunctionType.Copy)
                    h_p = pp_h.tile([P, D], F32, tag="h")
                    nc.tensor.matmul(h_p[:], lhsT=aggT_s[:], rhs=wt[:], start=True, stop=False)
                    nc.tensor.matmul(h_p[:], lhsT=mask_t[:1, w * P:(w + 1) * P], rhs=br[:],
                                     start=False, stop=True)
                    h_s = wp.tile([P, D], table_dt, tag="hs")
                    nc.scalar.activation(h_s[:], h_p[:], mybir.ActivationFunctionType.Relu)
                    if layer == 0:
                        nc.sync.dma_start(h_loc[w * P:(w + 1) * P, :], h_s[:])
                    else:
                        lg = w // WPG
                        hT_p = pp_p.tile([P, P], BF16, tag="hT")
                        nc.tensor.transpose(hT_p[:], h_s[:], identb_t[:])
                        wmax = wp.tile([P, 1], F32, tag="wmax")
                        nc.vector.reduce_max(wmax[:], hT_p[:], axis=mybir.AxisListType.X)
                        nc.vector.tensor_tensor(out=poolT[:, lg:lg + 1], in0=wmax[:],
                                                in1=poolT[:, lg:lg + 1],
                                                op=mybir.AluOpType.max)
            if layer == 0:
                nc.gpsimd.collective_compute(
                    "AllGather", mybir.AluOpType.bypass,
                    replica_groups=[list(range(NC))],
                    ins=[h_loc[:]], outs=[h_tab[:]],
                )
        if debug_taps:
            hb = wp.tile([P, D], F32, tag="hdbg")
            for w in range(WIN):
                nc.gpsimd.dma_start(hb[:], h_loc[w * P:(w + 1) * P, :])
                nc.sync.dma_start(dbg_h[w * P:(w + 1) * P, :], hb[:])

        # ---- head ----
        NGr = NC * GPC
        nc.sync.dma_start(pag_in[:], poolT[:])
        nc.gpsimd.collective_compute(
            "AllGather", mybir.AluOpType.bypass,
            replica_groups=[list(range(NC))],
            ins=[pag_in[:]], outs=[pag_out[:]],
        )
        pall = cp.tile([P, NGr], F32)
        pr = pag_out[:].rearrange("(c p) g -> p c g", c=NC)
        for c in range(NC):
            nc.sync.dma_start(pall[:, c * GPC:(c + 1) * GPC], pr[:, c, :])
        if debug_taps:
            nc.sync.dma_start(dbg_pool[:], pall[:])
        z_p = pp_agg.tile([P, NGr], F32, tag="agg")
        nc.tensor.matmul(z_p[:], lhsT=wf1t_t[:], rhs=pall[:], start=True, stop=False)
        nc.tensor.matmul(z_p[:], lhsT=bf1r_t[:1, :], rhs=onesg_t[:1, :], start=False, stop=True)
        zr = wp.tile([P, NGr], F32, tag="zr")
        nc.scalar.activation(zr[:], z_p[:], mybir.ActivationFunctionType.Relu)
        z2_p = pp_h.tile([OUT, NGr], F32, tag="h")
        nc.tensor.matmul(z2_p[:], lhsT=wf2t_t[:], rhs=zr[:], start=True, stop=False)
        nc.tensor.matmul(z2_p[:], lhsT=bf2r_t[:1, :], rhs=onesg_t[:1, :], start=False, stop=True)
        z2 = wp.tile([OUT, NGr], F32, tag="z2")
        nc.vector.tensor_copy(z2[:], z2_p[:])
        zt_p = pp_t.tile([NGr, OUT], F32, tag="aggT")
        nc.tensor.transpose(zt_p[:], z2[:], identf_t[:OUT, :OUT])
        zt = wp.tile([NGr, OUT], F32, tag="zt")
        nc.vector.tensor_copy(zt[:], zt_p[:])
        mx = wp.tile([NGr, 1], F32, tag="mx")
        nc.vector.reduce_max(mx[:], zt[:], axis=mybir.AxisListType.X)
        zs = wp.tile([NGr, OUT], F32, tag="zs")
        nc.vector.tensor_scalar(out=zs[:], in0=zt[:], scalar1=mx[:], scalar2=None,
                                op0=mybir.AluOpType.subtract)
        ex = wp.tile([NGr, OUT], F32, tag="ex")
        nc.scalar.activation(ex[:], zs[:], mybir.ActivationFunctionType.Exp)
        sm = wp.tile([NGr, 1], F32, tag="sm")
        nc.vector.reduce_sum(sm[:], ex[:], axis=mybir.AxisListType.X)
        lg_ = wp.tile([NGr, 1], F32, tag="lg")
        nc.scalar.activation(lg_[:], sm[:], mybir.ActivationFunctionType.Ln)
        logz = wp.tile([NGr, 1], F32, tag="logz")
        nc.vector.tensor_tensor(out=logz[:], in0=mx[:], in1=lg_[:],
                                op=mybir.AluOpType.add)
        res = wp.tile([NGr, OUT], F32, tag="res")
        nc.vector.tensor_scalar(out=res[:], in0=zt[:], scalar1=logz[:], scalar2=None,
                                op0=mybir.AluOpType.subtract)
        nc.sync.dma_start(out[:], res[:])

        for p_ in (dp, pp_p, pp_h, pp_t, pp_agg, ohp, mp, wp, cp):
            p_.release()
    nc.compile()
    return nc


def make_inputs(st, percore, W1, b1, W2, b2, Wf1, bf1, Wf2, bf2, table_np=np.float32):
    NC, GPC = st["NC"], st["GPC"]
    bf = ml_dtypes.bfloat16
    iota = np.broadcast_to(np.arange(P, dtype=np.float32), (P, P)).astype(bf)
    ident = np.eye(P, dtype=np.float32)
    common = dict(
        xt=percore["xt"],
        w1t=np.ascontiguousarray(np.asarray(W1, np.float32).T).astype(bf),
        w2t=np.ascontiguousarray(np.asarray(W2, np.float32).T).astype(bf),
        b1r=np.asarray(b1, np.float32)[None, :].astype(bf),
        b2r=np.asarray(b2, np.float32)[None, :].astype(bf),
        wf1t=np.ascontiguousarray(np.asarray(Wf1, np.float32).T),
        bf1r=np.asarray(bf1, np.float32)[None, :],
        wf2t=np.ascontiguousarray(np.asarray(Wf2, np.float32).T),
        bf2r=np.asarray(bf2, np.float32)[None, :],
        iota=np.ascontiguousarray(iota),
        identb=ident.astype(bf),
        identf=ident,
        onesg=np.ones((1, NC * GPC), np.float32),
    )
    in_maps = []
    for c in range(NC):
        m = dict(common)
        m["idx16"] = np.ascontiguousarray(percore["idx16"][c])
        m["dstl"] = np.ascontiguousarray(percore["dst"][c])
        m["recip"] = np.ascontiguousarray(percore["recip"][c])
        m["mask"] = np.ascontiguousarray(percore["mask"][c])
        in_maps.append(m)
    return in_maps


_CACHE = {}


def kernel(**inputs):
    """Full-input GNN kernel: shards across 8 NeuronCores internally."""
    import os
    x = np.asarray(inputs["x"], np.float32)
    ei = np.asarray(inputs["edge_index"])
    batch = np.asarray(inputs["batch"])
    st, percore, _meta = preprocess(x, ei, batch)
    key = (st["WIN"], st["NPC"], st["S_tot"], st["NQ"])
    if key not in _CACHE:
        _CACHE[key] = build_nc(st)
    nc = _CACHE[key]
    in_maps = make_inputs(st, percore,
                          inputs["W1"], inputs["b1"], inputs["W2"], inputs["b2"],
                          inputs["Wf1"], inputs["bf1"], inputs["Wf2"], inputs["bf2"])
    trace = os.environ.get("GNN_TRACE", "0") == "1"
    res = run_bass_kernel_spmd(nc, in_maps, core_ids=list(range(st["NC"])), trace=trace)
    global LAST_EXEC_NS, LAST_TRACE
    LAST_EXEC_NS = res.exec_time_ns
    LAST_TRACE = res.instructions_and_trace[1] if res.instructions_and_trace else None
    return np.asarray(res.results[0]["out"], np.float32)


LAST_EXEC_NS = None
LAST_TRACE = None



# revision 2
# speedup vs baseline: 1.0001x; 1.0001x over previous
"""GNN message-passing kernel v3 for trn2: pair-window feature-major aggregation,
chunked AllGather overlap, per-core head. Both layers pair-major."""
import os
import numpy as np
import ml_dtypes
import concourse.bass as bass
import concourse.tile as tile
from concourse import bacc, mybir
from concourse.bass_utils import run_bass_kernel_spmd

F32 = mybir.dt.float32
BF16 = mybir.dt.bfloat16
I16 = mybir.dt.int16
P = 128
D = 128
OUT = 2
NC = 8
NCHUNK = 4
MAXSUB = 8          # 1024-desc ring limit per SWDGE call
bf = ml_dtypes.bfloat16


def preprocess(x, edge_index, batch):
    x = np.asarray(x, np.float32)
    ei = np.asarray(edge_index, np.int64)
    b = np.asarray(batch, np.int64)
    N = x.shape[0]
    G = int(b.max()) + 1
    GPC = G // NC
    counts = np.bincount(b, minlength=G)
    assert counts.min() > 0
    WPG = int(np.ceil(counts.max() / P))
    NPG = WPG * P
    WIN = GPC * WPG
    NPC = WIN * P
    assert WIN % (2 * NCHUNK) == 0, WIN
    WCH = WIN // NCHUNK
    CR = WCH * P
    TR = NC * CR
    assert TR <= 32767
    NPAIR = WIN // 2
    PPCH = NPAIR // NCHUNK

    cum = np.concatenate([[0], np.cumsum(counts)])
    base_new = (np.arange(G) // GPC) * NPC + (np.arange(G) % GPC) * NPG
    perm = base_new[b] + (np.arange(N) - cum[b])

    # chunk-major table layout: row(n) = k*TR + c*CR + rr
    c_of = perm // NPC
    loc = perm % NPC
    rowmap = (loc // CR) * TR + c_of * CR + loc % CR
    xt = np.zeros((NCHUNK * TR, D), bf)
    xt[rowmap] = x.astype(bf)

    src = perm[ei[0]]
    dst = perm[ei[1]]
    deg = np.bincount(dst, minlength=NC * NPC)
    recip_node = (1.0 / np.maximum(deg, 1)).astype(np.float32)

    core = dst // NPC
    w = (dst % NPC) // P
    dl = dst % P
    wp = w // 2
    sk = (src % NPC) // CR
    srow = (src // NPC) * CR + (src % NPC) % CR

    key = (core * WIN + w) * NCHUNK + sk
    order = np.argsort(key, kind="stable")
    s_row = srow[order]
    s_q = dl[order].astype(np.float32)
    L = np.bincount(key[order], minlength=NC * WIN * NCHUNK).reshape(
        NC, WIN, NCHUNK)
    S = np.ceil(L / P).astype(np.int64).max(axis=0)        # [WIN, NCHUNK]
    S_tot = int(S.sum())
    S_win = S.sum(axis=1)                                  # [WIN]
    S_pair = S_win[0::2] + S_win[1::2]                     # [NPAIR]

    sub0 = np.zeros((WIN, NCHUNK), np.int64)
    run = 0
    for w_ in range(WIN):
        for k in range(NCHUNK):
            sub0[w_, k] = run
            run += S[w_, k]
    assert run == S_tot

    grp_start = np.concatenate([[0], np.cumsum(L.reshape(-1))])
    idx_a = np.zeros((NC, S_tot * P), np.int64)
    q_a = np.full((NC, S_tot * P), 300.0, np.float32)
    for c in range(NC):
        for w_ in range(WIN):
            for k in range(NCHUNK):
                n = L[c, w_, k]
                if n == 0:
                    continue
                a = grp_start[(c * WIN + w_) * NCHUNK + k]
                base = sub0[w_, k] * P
                idx_a[c, base:base + n] = s_row[a:a + n]
                q_a[c, base:base + n] = s_q[a:a + n]
    assert idx_a.max() < TR and idx_a.min() >= 0

    idx16 = np.ascontiguousarray(
        idx_a.reshape(NC, S_tot * 8, 16).transpose(0, 2, 1)).astype(np.int16)
    idx16 = np.tile(idx16, (1, 8, 1))
    dst16 = np.ascontiguousarray(
        q_a.reshape(NC, S_tot, P).transpose(0, 2, 1)).astype(bf)

    valid = np.zeros((NC * NPC,), np.float32)
    for g in range(G):
        base = (g // GPC) * NPC + (g % GPC) * NPG
        valid[base:base + counts[g]] = 1.0
    degc = np.maximum(deg, 1).astype(np.float32)
    assert degc.max() < 256
    maskdeg = (valid * degc).astype(bf).reshape(NC, 1, NPC)
    recwin = np.ascontiguousarray(
        recip_node.reshape(NC, WIN, P).transpose(0, 2, 1)).astype(np.float32)

    st = dict(NC=NC, G=G, GPC=GPC, WPG=WPG, WIN=WIN, NPC=NPC, CR=CR, TR=TR,
              NPAIR=NPAIR, PPCH=PPCH, S=S, S_pair=S_pair, S_win=S_win,
              sub0=sub0, S_tot=S_tot)
    percore = dict(idx16=idx16, dst16=dst16, maskdeg=maskdeg, recwin=recwin, xt=xt)
    return st, percore


def build_nc(st):
    NPAIR, S, S_tot, sub0 = st["NPAIR"], st["S"], st["S_tot"], st["sub0"]
    S_win = st["S_win"]
    TR, CR, GPC, WPG, NPC = st["TR"], st["CR"], st["GPC"], st["WPG"], st["NPC"]
    PPCH, WIN = st["PPCH"], st["WIN"]
    S_pair = st["S_pair"]
    Spair_max = int(S_pair.max())
    Scell_max = int(S.max())
    assert Scell_max <= 16, Scell_max

    nc = bacc.Bacc("TRN2", target_bir_lowering=False, debug=False,
                   num_devices=NC, num_swdge_queues=4,
                   dynamic_dma_scratch_size=16384)
    xt_in = nc.dram_tensor("xt", [NCHUNK * TR, D], BF16, kind="ExternalInput")
    i0_in = nc.dram_tensor("idx16", [P, S_tot * 8], I16, kind="ExternalInput")
    d0_in = nc.dram_tensor("dst16", [P, S_tot], BF16, kind="ExternalInput")
    mask_in = nc.dram_tensor("maskdeg", [1, NPC], BF16, kind="ExternalInput")
    rw_in = nc.dram_tensor("recwin", [P, WIN], F32, kind="ExternalInput")
    w1t_in = nc.dram_tensor("w1t", [D, D], BF16, kind="ExternalInput")
    w2t_in = nc.dram_tensor("w2t", [D, D], BF16, kind="ExternalInput")
    b1r_in = nc.dram_tensor("b1r", [1, D], BF16, kind="ExternalInput")
    b2r_in = nc.dram_tensor("b2r", [1, D], BF16, kind="ExternalInput")
    wf1t_in = nc.dram_tensor("wf1t", [D, D], F32, kind="ExternalInput")
    bf1r_in = nc.dram_tensor("bf1r", [1, D], F32, kind="ExternalInput")
    wf2t_in = nc.dram_tensor("wf2t", [D, OUT], F32, kind="ExternalInput")
    bf2r_in = nc.dram_tensor("bf2r", [1, OUT], F32, kind="ExternalInput")
    iota1_in = nc.dram_tensor("iota1", [P, P], BF16, kind="ExternalInput")
    identb_in = nc.dram_tensor("identb", [P, P], BF16, kind="ExternalInput")
    identf_in = nc.dram_tensor("identf", [P, P], F32, kind="ExternalInput")
    onesg_in = nc.dram_tensor("onesg", [1, GPC], F32, kind="ExternalInput")
    out = nc.dram_tensor("out", [GPC, OUT], F32, kind="ExternalOutput")
    dbg = os.environ.get("GNN_DBG", "0") == "1"
    if dbg:
        dbg_h = nc.dram_tensor("dbg_h", [NPC, D], BF16, kind="ExternalOutput")
        dbg_pool = nc.dram_tensor("dbg_pool", [P, GPC], F32, kind="ExternalOutput")

    with tile.TileContext(nc) as tc:
        cp = tc.alloc_tile_pool(name="const", bufs=1)
        mp = tc.alloc_tile_pool(name="msgs", bufs=4)
        op_ = tc.alloc_tile_pool(name="oh", bufs=12)
        wp_ = tc.alloc_tile_pool(name="work", bufs=4)
        pp_agg = tc.alloc_tile_pool(name="ps_agg", bufs=3, space="PSUM")
        pp_h = tc.alloc_tile_pool(name="ps_h", bufs=3, space="PSUM")
        pp_c = tc.alloc_tile_pool(name="ps_c", bufs=2, space="PSUM")
        dp = tc.alloc_tile_pool(name="dram", bufs=1, space="DRAM")

        h_loc = dp.tile([NPC, D], BF16)
        hc = [dp.tile([TR, D], BF16, addr_space="Shared", name=f"hc{i}")
              for i in range(NCHUNK)]

        d0_t = cp.tile([P, S_tot], BF16)
        nc.sync.dma_start(d0_t[:], d0_in[:])
        i0_t = cp.tile([P, S_tot * 8], I16)
        nc.sync.dma_start(i0_t[:], i0_in[:])
        mask_t = cp.tile([1, NPC], BF16)
        nc.sync.dma_start(mask_t[:], mask_in[:])
        recw_t = cp.tile([P, WIN], F32)
        nc.sync.dma_start(recw_t[:], rw_in[:])
        w1t_t = cp.tile([D, D], BF16)
        nc.sync.dma_start(w1t_t[:], w1t_in[:])
        w2t_t = cp.tile([D, D], BF16)
        nc.sync.dma_start(w2t_t[:], w2t_in[:])
        b1r_t = cp.tile([1, D], BF16)
        nc.sync.dma_start(b1r_t[:], b1r_in[:])
        b2r_t = cp.tile([1, D], BF16)
        nc.sync.dma_start(b2r_t[:], b2r_in[:])
        wf1t_t = cp.tile([D, D], F32)
        nc.sync.dma_start(wf1t_t[:], wf1t_in[:])
        bf1r_t = cp.tile([1, D], F32)
        nc.sync.dma_start(bf1r_t[:], bf1r_in[:])
        wf2t_t = cp.tile([D, OUT], F32)
        nc.sync.dma_start(wf2t_t[:], wf2t_in[:])
        bf2r_t = cp.tile([1, OUT], F32)
        nc.sync.dma_start(bf2r_t[:], bf2r_in[:])
        iota1_t = cp.tile([P, P], BF16)
        nc.sync.dma_start(iota1_t[:], iota1_in[:])
        identb_t = cp.tile([P, P], BF16)
        nc.sync.dma_start(identb_t[:], identb_in[:])
        identf_t = cp.tile([P, P], F32)
        nc.sync.dma_start(identf_t[:], identf_in[:])
        onesg_t = cp.tile([1, GPC], F32)
        nc.sync.dma_start(onesg_t[:], onesg_in[:])
        poolT = cp.tile([P, GPC], F32)
        nc.vector.memset(poolT[:], 0)

        qrr = [0]

        def gather_cell(table_k, w_, k, m3, moff):
            Sc = int(S[w_, k])
            if Sc == 0:
                return 0
            c0 = int(sub0[w_, k])
            done = 0
            nsplit = (Sc + MAXSUB - 1) // MAXSUB
            per = (Sc + nsplit - 1) // nsplit
            while done < Sc:
                n = min(per, Sc - done)
                nc.gpsimd.dma_gather(
                    out_ap=m3[:, moff + done:moff + done + n, :],
                    in_ap=table_k[:, :],
                    idxs_ap=i0_t[:, (c0 + done) * 8:(c0 + done + n) * 8],
                    num_idxs=n * P, num_idxs_reg=n * P,
                    elem_size=D, queue_num=qrr[0] % 4)
                qrr[0] += 1
                done += n
            return Sc

        def build_oh_cell(c0, Sc):
            oh = op_.tile([P, Sc * P], BF16, tag="oh",
                          padded_shape=[P, Scell_max * P])
            oh3 = oh[:].rearrange("p (s q) -> p s q", q=P)
            nc.vector.tensor_tensor(
                out=oh3[:, :, :],
                in0=d0_t[:, c0:c0 + Sc].rearrange(
                    "p (s o) -> p s o", o=1).to_broadcast([P, Sc, P]),
                in1=iota1_t[:].rearrange(
                    "(o p) q -> p o q", o=1).to_broadcast([P, Sc, P]),
                op=mybir.AluOpType.is_equal)
            return oh3

        def pair_agg(p_, layer):
            """Gathers + one-hots + agg matmuls for pair p_. Returns aggs tile."""
            Sp = int(S_pair[p_])
            msgs = mp.tile([P, Sp * D], BF16, tag=f"m{layer}",
                           padded_shape=[P, Spair_max * D])
            m3 = msgs[:].rearrange("p (s d) -> p s d", d=D)
            moff = 0
            offs = {}
            for half in range(2):
                w = 2 * p_ + half
                for k in range(NCHUNK):
                    offs[(half, k)] = moff
                    tab = (xt_in[k * TR:(k + 1) * TR, :] if layer == 0
                           else hc[k][:, :])
                    moff += gather_cell(tab, w, k, m3, moff)
            agg = pp_agg.tile([P, 2 * P], F32, tag="agg", padded_shape=[P, 2 * P])
            mm = {0: [], 1: []}
            for half in range(2):
                w = 2 * p_ + half
                for k in range(NCHUNK):
                    Sc = int(S[w, k])
                    if Sc == 0:
                        continue
                    oh3 = build_oh_cell(int(sub0[w, k]), Sc)
                    for s in range(Sc):
                        mm[half].append((offs[(half, k)] + s, oh3, s))
            nmax = max(len(mm[0]), len(mm[1]))
            for i in range(nmax):
                for half in range(2):
                    if i < len(mm[half]):
                        mcol, oh3, s = mm[half][i]
                        nc.tensor.matmul(
                            agg[:, half * P:(half + 1) * P],
                            lhsT=m3[:, mcol, :],
                            rhs=oh3[:, s, :], start=(i == 0),
                            stop=(i == len(mm[half]) - 1))
            aggs = wp_.tile([P, 2 * P], BF16, tag="aggs")
            nc.scalar.activation(aggs[:], agg[:],
                                 mybir.ActivationFunctionType.Copy)
            return aggs

        # ================= layer 0 =================
        pend = []

        def l0_tail():
            p0, aggs0 = pend.pop(0)
            for half in range(2):
                w = 2 * p0 + half
                if int(S_win[w]) == 0:
                    continue
                h_ps = pp_h.tile([P, D], F32, tag="h", padded_shape=[P, D])
                nc.tensor.matmul(h_ps[:], lhsT=aggs0[:, half * P:(half + 1) * P],
                                 rhs=w1t_t[:], start=True, stop=False)
                nc.tensor.matmul(h_ps[:], lhsT=mask_t[:1, w * P:(w + 1) * P],
                                 rhs=b1r_t[:1, :], start=False, stop=True)
                h_s = wp_.tile([P, D], BF16, tag="hs")
                nc.scalar.activation(h_s[:], h_ps[:],
                                     mybir.ActivationFunctionType.Relu,
                                     scale=recw_t[:, w:w + 1])
                nc.scalar.dma_start(h_loc[w * P:(w + 1) * P, :], h_s[:])

        for p_ in range(NPAIR):
            if int(S_pair[p_]) > 0:
                aggs = pair_agg(p_, 0)
                pend.append((p_, aggs))
                if len(pend) > 1:
                    l0_tail()
            if (p_ + 1) % PPCH == 0:
                while pend:
                    l0_tail()
                kc = (p_ + 1) // PPCH - 1
                nc.gpsimd.collective_compute(
                    "AllGather", mybir.AluOpType.bypass,
                    replica_groups=[list(range(NC))],
                    ins=[h_loc[kc * CR:(kc + 1) * CR, :]], outs=[hc[kc][:]])
        while pend:
            l0_tail()

        if dbg:
            for w in range(WIN):
                hb = wp_.tile([P, D], BF16, tag="hdbg")
                nc.sync.dma_start(hb[:], h_loc[w * P:(w + 1) * P, :])
                nc.sync.dma_start(dbg_h[w * P:(w + 1) * P, :], hb[:])

        # ================= layer 1 =================
        pend2 = []

        def l1_tail():
            p0, aggs0 = pend2.pop(0)
            for half in range(2):
                w = 2 * p0 + half
                if int(S_win[w]) == 0:
                    continue
                lg = w // WPG
                h2_ps = pp_h.tile([P, D], F32, tag="h", padded_shape=[P, D])
                nc.tensor.matmul(h2_ps[:], lhsT=aggs0[:, half * P:(half + 1) * P],
                                 rhs=w2t_t[:], start=True, stop=False)
                nc.tensor.matmul(h2_ps[:], lhsT=mask_t[:1, w * P:(w + 1) * P],
                                 rhs=b2r_t[:1, :], start=False, stop=True)
                h2_s = wp_.tile([P, D], BF16, tag="hs")
                nc.scalar.activation(h2_s[:], h2_ps[:],
                                     mybir.ActivationFunctionType.Relu,
                                     scale=recw_t[:, w:w + 1])
                hT_ps = pp_c.tile([P, P], BF16, tag="aggc", padded_shape=[P, 2 * P])
                nc.tensor.transpose(hT_ps[:], h2_s[:], identb_t[:])
                wmax = wp_.tile([P, 1], F32, tag="wmax")
                nc.vector.reduce_max(wmax[:], hT_ps[:], axis=mybir.AxisListType.X)
                nc.vector.tensor_tensor(out=poolT[:, lg:lg + 1], in0=wmax[:],
                                        in1=poolT[:, lg:lg + 1],
                                        op=mybir.AluOpType.max)

        for p_ in range(NPAIR):
            if int(S_pair[p_]) == 0:
                continue
            aggs = pair_agg(p_, 1)
            pend2.append((p_, aggs))
            if len(pend2) > 1:
                l1_tail()
        while pend2:
            l1_tail()

        if dbg:
            nc.sync.dma_start(dbg_pool[:], poolT[:])

        # ================= head (local graphs only) =================
        z1_ps = pp_agg.tile([P, GPC], F32, tag="agg", padded_shape=[P, 2 * P])
        nc.tensor.matmul(z1_ps[:], lhsT=wf1t_t[:], rhs=poolT[:], start=True, stop=False)
        nc.tensor.matmul(z1_ps[:], lhsT=bf1r_t[:1, :], rhs=onesg_t[:1, :],
                         start=False, stop=True)
        z1 = wp_.tile([P, GPC], F32, tag="z1")
        nc.scalar.activation(z1[:], z1_ps[:], mybir.ActivationFunctionType.Relu)
        z2_ps = pp_h.tile([OUT, GPC], F32, tag="h", padded_shape=[P, D])
        nc.tensor.matmul(z2_ps[:], lhsT=wf2t_t[:], rhs=z1[:], start=True, stop=False)
        nc.tensor.matmul(z2_ps[:], lhsT=bf2r_t[:1, :], rhs=onesg_t[:1, :],
                         start=False, stop=True)
        z2 = wp_.tile([OUT, GPC], F32, tag="z2")
        nc.vector.tensor_copy(z2[:], z2_ps[:])
        zt_ps = pp_c.tile([GPC, OUT], F32, tag="aggc", padded_shape=[P, 2 * P])
        nc.tensor.transpose(zt_ps[:], z2[:], identf_t[:OUT, :OUT])
        zt = wp_.tile([GPC, OUT], F32, tag="zt")
        nc.vector.tensor_copy(zt[:], zt_ps[:])
        mx = wp_.tile([GPC, 1], F32, tag="mx")
        nc.vector.reduce_max(mx[:], zt[:], axis=mybir.AxisListType.X)
        zs = wp_.tile([GPC, OUT], F32, tag="zs")
        nc.vector.tensor_scalar(out=zs[:], in0=zt[:], scalar1=mx[:], scalar2=None,
                                op0=mybir.AluOpType.subtract)
        ex = wp_.tile([GPC, OUT], F32, tag="ex")
        nc.scalar.activation(ex[:], zs[:], mybir.ActivationFunctionType.Exp)
        sm = wp_.tile([GPC, 1], F32, tag="sm")
        nc.vector.reduce_sum(sm[:], ex[:], axis=mybir.AxisListType.X)
        lg_ = wp_.tile([GPC, 1], F32, tag="lg")
        nc.scalar.activation(lg_[:], sm[:], mybir.ActivationFunctionType.Ln)
        logz = wp_.tile([GPC, 1], F32, tag="logz")
        nc.vector.tensor_tensor(out=logz[:], in0=mx[:], in1=lg_[:],
                                op=mybir.AluOpType.add)
        res = wp_.tile([GPC, OUT], F32, tag="res")
        nc.vector.tensor_scalar(out=res[:], in0=zt[:], scalar1=logz[:], scalar2=None,
                                op0=mybir.AluOpType.subtract)
        nc.sync.dma_start(out[:], res[:])

        for pool in (dp, pp_c, pp_h, pp_agg, wp_, op_, mp, cp):
            pool.release()
    nc.compile()
    return nc


def make_inputs(st, percore, W1, b1, W2, b2, Wf1, bf1, Wf2, bf2):
    GPC = st["GPC"]
    iota2 = np.broadcast_to(np.arange(2 * P, dtype=np.float32), (P, 2 * P))
    ident = np.eye(P, dtype=np.float32)
    common = dict(
        xt=percore["xt"],
        w1t=np.ascontiguousarray(np.asarray(W1, np.float32).T).astype(bf),
        w2t=np.ascontiguousarray(np.asarray(W2, np.float32).T).astype(bf),
        b1r=np.asarray(b1, np.float32)[None, :].astype(bf),
        b2r=np.asarray(b2, np.float32)[None, :].astype(bf),
        wf1t=np.ascontiguousarray(np.asarray(Wf1, np.float32).T),
        bf1r=np.asarray(bf1, np.float32)[None, :],
        wf2t=np.ascontiguousarray(np.asarray(Wf2, np.float32).T),
        bf2r=np.asarray(bf2, np.float32)[None, :],
        iota1=np.ascontiguousarray(iota2[:, :P].astype(bf)),
        identb=ident.astype(bf),
        identf=ident,
        onesg=np.ones((1, GPC), np.float32),
    )
    in_maps = []
    for c in range(NC):
        m = dict(common)
        m["idx16"] = np.ascontiguousarray(percore["idx16"][c])
        m["dst16"] = np.ascontiguousarray(percore["dst16"][c])
        m["maskdeg"] = np.ascontiguousarray(percore["maskdeg"][c])
        m["recwin"] = np.ascontiguousarray(percore["recwin"][c])
        in_maps.append(m)
    return in_maps


_CACHE = {}
LAST_EXEC_NS = None
LAST_TRACE = None
LAST_RESULTS = None


def kernel(**inputs):
    x = np.asarray(inputs["x"], np.float32)
    ei = np.asarray(inputs["edge_index"])
    batch = np.asarray(inputs["batch"])
    st, percore = preprocess(x, ei, batch)
    key = (st["WIN"], st["NPC"], st["S_tot"])
    if key not in _CACHE:
        _CACHE[key] = build_nc(st)
    nc = _CACHE[key]
    in_maps = make_inputs(st, percore,
                          inputs["W1"], inputs["b1"], inputs["W2"], inputs["b2"],
                          inputs["Wf1"], inputs["bf1"], inputs["Wf2"], inputs["bf2"])
    trace = os.environ.get("GNN_TRACE", "0") == "1"
    res = run_bass_kernel_spmd(nc, in_maps, core_ids=list(range(NC)), trace=trace)
    global LAST_EXEC_NS, LAST_TRACE, LAST_RESULTS
    LAST_EXEC_NS = res.exec_time_ns
    LAST_TRACE = res.instructions_and_trace[1] if res.instructions_and_trace else None
    LAST_RESULTS = res.results
    return np.concatenate(
        [np.asarray(res.results[c]["out"], np.float32) for c in range(NC)], axis=0)
